# revision 7
# baseline (speedup 1.0000x reference)
"""Trainium2 Bass kernel for the CustomGATLayer problem.

Strategy: data-parallel over batch B=8 across the 8 NeuronCores (one batch
element per core).  Per core the masked GAT attention

    P[j,i] = adj_T[j,i] * exp(LeakyReLU(e1[i] + e2[j], 0.2))

is computed in a transposed layout (source node j on partitions, query node
i on the free axis) via two complementary routes, chosen per (head, j-tile)
to balance engine load:

* Route A (PE+ACT):  the PE builds  S = 1 (x) e1  +  I @ (-60*(1-adj_T))
  directly in PSUM (rank-1 broadcast matmul plus an identity-stationary
  "mask add" matmul), so the adjacency mask is folded additively into the
  scores at zero vector-engine cost.  The ACT engine then applies
  Prelu(S + e2[j], alpha=.2) and Exp.  Masked entries come out as
  exp(.2*(s-60)) ~ 3e-5, which is negligible against typical row masses.

* Route D (DVE/Pool):  exp factorisation  P = adj_T * max(exp(e1)exp(e2),
  exp(.2 e1)exp(.2 e2)) using exp'd rank-1 factors: a DMA-broadcast row
  tensor V1b/V2b (from a DRAM round-trip) times per-partition scalars, a
  tensor_tensor max (mostly on GPSIMD/Pool) and the adjacency mask multiply
  (DVE).

The ones-column and the output bias are folded into the Wh matmul by
augmenting x with a ones row and W with a bias row, so the softmax
denominator rides along as matmul column 32 and the epilogue is a single
fused (x*recip) relu-max per (head, tile).  Two heads (one per route) are
processed as a pair sharing a [66, N] PSUM accumulator; the epilogue
transposes both heads' results in one PE transpose.
"""

import numpy as np

import concourse.bacc as bacc
import concourse.bass as bass
import concourse.mybir as mybir
import concourse.tile as tile
from concourse.bass_utils import run_bass_kernel_spmd
from concourse.masks import make_identity

B, N, D, H, HD = 8, 1024, 256, 8, 32
NT = N // 128  # node tiles per core
WC = H * (HD + 1) + 2 * H  # 280 = 264 wh(+ones) cols + 8 e1 cols + 8 e2 cols
F32 = mybir.dt.float32
F16 = mybir.dt.float16
AL = mybir.AluOpType
AF = mybir.ActivationFunctionType

# (A-route head, D-route head, #A-tiles of the A head) per processing pair.
PAIRS = [(5, 0, 7), (6, 1, 7), (7, 2, 6), (4, 3, 6)]

MASKV = -60.0  # additive mask: exp(0.2*(s-60)) ~ 0 at f16

_CACHE: dict = {}


def _build_bass():
    nc = bacc.Bacc("TRN2", target_bir_lowering=False, debug=False, num_devices=B)

    xT_d = nc.dram_tensor("xT", [D + 1, N], F16, kind="ExternalInput")
    wc_d = nc.dram_tensor("wc", [D + 1, WC], F16, kind="ExternalInput")
    adjT_d = nc.dram_tensor("adjT", [N, N], F16, kind="ExternalInput")
    nadj_d = nc.dram_tensor("nadj", [N, N], F16, kind="ExternalInput")
    out_d = nc.dram_tensor("out", [N, H * HD], F32, kind="ExternalOutput")
    vsc_d = nc.dram_tensor("vsc", [2 * H, N], F16, kind="Internal")

    with tile.TileContext(nc) as tc:
        with (
            tc.tile_pool(name="cst", bufs=1) as cst,
            tc.tile_pool(name="wrk", bufs=3) as wrk,
            tc.tile_pool(name="pp", bufs=8) as pp,
            tc.tile_pool(name="pst", bufs=2, space=bass.MemorySpace.PSUM) as pst,
            tc.tile_pool(name="psS", bufs=2, space=bass.MemorySpace.PSUM) as psS,
            tc.tile_pool(name="psV", bufs=1, space=bass.MemorySpace.PSUM) as psV,
        ):
            xt = [cst.tile([128, N], F16, tag=f"xt{k}", name=f"xt{k}") for k in range(2)]
            xta = cst.tile([1, N], F16, tag="xta", name="xta")
            wc = [cst.tile([128, WC], F16, tag=f"wc{k}", name=f"wc{k}") for k in range(2)]
            wca = cst.tile([1, WC], F16, tag="wca", name="wca")
            adjT = [cst.tile([128, N], F16, tag=f"adj{j}", name=f"adj{j}") for j in range(NT)]
            nadj = [cst.tile([128, N], F16, tag=f"nad{j}", name=f"nad{j}") for j in range(NT)]
            whb = [cst.tile([128, H * (HD + 1)], F16, tag=f"whb{j}", name=f"whb{j}") for j in range(NT)]
            e12a = cst.tile([128, 16 * NT], F32, tag="e12a", name="e12a")
            e2a = cst.tile([128, H * NT], F32, tag="e2a", name="e2a")
            ue1 = cst.tile([128, H * NT], F32, tag="ue1", name="ue1")
            ue2 = cst.tile([128, H * NT], F32, tag="ue2", name="ue2")
            eiT16 = cst.tile([H, N], F16, tag="eiT16", name="eiT16")
            e1row = [cst.tile([1, N], F16, tag=f"e1r{h}", name=f"e1r{h}") for h in range(H)]
            v1T = cst.tile([H, N], F16, tag="v1T", name="v1T")
            v2T = cst.tile([H, N], F16, tag="v2T", name="v2T")
            V1b = [cst.tile([128, N], F16, tag=f"V1b{h}", name=f"V1b{h}") for h in range(H)]
            V2b = [cst.tile([128, N], F16, tag=f"V2b{h}", name=f"V2b{h}") for h in range(H)]
            idf16 = cst.tile([128, 128], F16, tag="idf16", name="idf16")
            idf32 = cst.tile([128, 128], F32, tag="idf32", name="idf32")
            ones_r = cst.tile([1, 128], F16, tag="ones_r", name="ones_r")
            outsb = [cst.tile([128, H * HD], F32, tag=f"o{t}", name=f"o{t}") for t in range(NT)]

            # ---- input loads.  nadj on the Pool queue (fast issue; needed
            # from the first A-tile), the rest on SP.
            for k in range(2):
                nc.sync.dma_start(xt[k][:], xT_d[k * 128 : (k + 1) * 128, :])
                nc.sync.dma_start(wc[k][:], wc_d[k * 128 : (k + 1) * 128, :])
            nc.sync.dma_start(xta[:], xT_d[D : D + 1, :])
            nc.sync.dma_start(wca[:], wc_d[D : D + 1, :])
            for j in range(NT):
                nc.gpsimd.dma_start(nadj[j][:], nadj_d[j * 128 : (j + 1) * 128, :])
            for j in range(NT):
                nc.sync.dma_start(adjT[j][:], adjT_d[j * 128 : (j + 1) * 128, :])
            make_identity(nc, idf16[:])
            make_identity(nc, idf32[:])
            nc.gpsimd.memset(ones_r[:], 1.0)

            # ---- per-head linear transform Wh (+ones col, +bias via x-aug)
            # and attention logit columns e1/e2.
            for t in range(NT):
                tb = slice(t * 128, (t + 1) * 128)
                whp = pst.tile([128, WC], F32, tag="tp", name="whp_t")
                nc.tensor.matmul(whp[:], xt[0][:, tb], wc[0][:], start=True, stop=False)
                nc.tensor.matmul(whp[:], xt[1][:, tb], wc[1][:], start=False, stop=False)
                nc.tensor.matmul(whp[:], xta[0:1, tb], wca[:], start=False, stop=True)
                nc.vector.tensor_copy(whb[t][:], whp[:, 0 : H * (HD + 1)])
                nc.vector.tensor_copy(e12a[:, t * 16 : (t + 1) * 16], whp[:, H * (HD + 1) : WC])
                nc.vector.tensor_copy(e2a[:, t * 8 : (t + 1) * 8], whp[:, H * (HD + 1) + H : WC])

            # exp'd e2 factors (per-partition scalars for route D)
            nc.scalar.activation(ue1[:], e2a[:], AF.Exp)
            nc.scalar.activation(ue2[:], e2a[:], AF.Exp, scale=0.2)

            # e1 rows: transpose per tile, keep f16 row form
            for t in range(NT):
                etp = pst.tile([16, 128], F32, tag="tp", name="etp_t")
                nc.tensor.transpose(etp[:], e12a[:, t * 16 : (t + 1) * 16], idf32[:])
                nc.vector.tensor_copy(eiT16[:, t * 128 : (t + 1) * 128], etp[0:H, :])

            # stage e1 rows at base partition 0 (PE moving operand requirement)
            for h in range(H):
                nc.gpsimd.dma_start(e1row[h][:], eiT16[h : h + 1, :])

            # exp'd e1 rows -> DRAM -> partition-broadcast tiles (route D)
            nc.scalar.activation(v1T[:], eiT16[:], AF.Exp)
            nc.scalar.activation(v2T[:], eiT16[:], AF.Exp, scale=0.2)
            nc.gpsimd.dma_start(vsc_d[0:H, :], v1T[:])
            nc.gpsimd.dma_start(vsc_d[H : 2 * H, :], v2T[:])
            for h in range(H):
                nc.gpsimd.dma_start(V1b[h][:], vsc_d[h : h + 1, :].partition_broadcast(128))
                nc.gpsimd.dma_start(V2b[h][:], vsc_d[H + h : H + h + 1, :].partition_broadcast(128))

            # ---- main attention loop: pairs of heads, tile-interleaved ----
            fin_rr = 0  # round-robin for the epilogue engine

            for pi, (hA, hD, nA) in enumerate(PAIRS):
                pvt2 = psV.tile([97, N], F32, tag="pv", name="pvt2_t")
                pend = []  # PV matmuls deferred one j-round for PE flow

                def flush_pv(pend=pend):
                    for h, srow, j, P in pend:
                        for half in range(2):
                            sl = slice(half * 512, (half + 1) * 512)
                            nc.tensor.matmul(
                                pvt2[srow : srow + 33, sl],
                                whb[j][:, h * 33 : (h + 1) * 33],
                                P[:, sl],
                                start=(j == 0),
                                stop=(j == NT - 1),
                            )
                    pend.clear()

                for j in range(NT):
                    prev = list(pend)
                    pend.clear()
                    for h, srow in ((hA, 0), (hD, 64)):
                        bias_ap = e12a[:, j * 16 + 8 + h : j * 16 + 9 + h]
                        if srow == 0 and j < nA:
                            # Route A: PE score build + ACT prelu/exp
                            S = psS.tile([128, N], F32, tag="S", name="S_t")
                            for half in range(2):
                                sl = slice(half * 512, (half + 1) * 512)
                                nc.tensor.matmul(
                                    S[:, sl], ones_r[:], e1row[h][0:1, sl],
                                    start=True, stop=False,
                                )
                                nc.tensor.matmul(
                                    S[:, sl], idf16[:], nadj[j][:, sl],
                                    start=False, stop=True,
                                )
                            lr = wrk.tile([128, N], F16, tag="lr", name="lr_t", bufs=3)
                            nc.scalar.activation(lr[:], S[:], AF.Prelu, bias=bias_ap, alpha=0.2)
                            P = pp.tile([128, N], F16, tag="P", name="P_t")
                            nc.scalar.activation(P[:], lr[:], AF.Exp)
                        else:
                            # Route D: exp'd rank-1 factors + max + mask
                            E1 = wrk.tile([128, N], F16, tag="E1", name="E1_t", bufs=3)
                            E2 = wrk.tile([128, N], F16, tag="E2", name="E2_t", bufs=3)
                            Pp = wrk.tile([128, N], F16, tag="Pp", name="Pp_t", bufs=3)
                            s1 = ue1[:, j * 8 + h : j * 8 + h + 1]
                            s2 = ue2[:, j * 8 + h : j * 8 + h + 1]
                            nc.vector.tensor_scalar(E1[:], V1b[h][:], s1, None, AL.mult)
                            nc.vector.tensor_scalar(E2[:], V2b[h][:], s2, None, AL.mult)
                            eng = nc.gpsimd if srow == 64 else nc.vector
                            eng.tensor_tensor(Pp[:], E1[:], E2[:], AL.max)
                            P = pp.tile([128, N], F16, tag="P", name="P_t")
                            nc.vector.tensor_tensor(P[:], Pp[:], adjT[j][:], AL.mult)
                        pend.append((h, srow, j, P))
                    # emit previous round's PV matmuls (keeps PE fed without
                    # head-of-line blocking on this round's P tiles)
                    for h, srow, jj, P in prev:
                        for half in range(2):
                            sl = slice(half * 512, (half + 1) * 512)
                            nc.tensor.matmul(
                                pvt2[srow : srow + 33, sl],
                                whb[jj][:, h * 33 : (h + 1) * 33],
                                P[:, sl],
                                start=(jj == 0),
                                stop=(jj == NT - 1),
                            )
                flush_pv()

                # ---- pair epilogue: drain accumulator, transpose, normalize
                pvs2 = wrk.tile([97, N], F32, tag="pvs", name="pvs_t", bufs=2)
                if pi == 0:
                    nc.gpsimd.tensor_copy(pvs2[:], pvt2[:])
                elif pi == 2:
                    nc.scalar.copy(pvs2[:], pvt2[:])
                else:
                    nc.vector.tensor_copy(pvs2[:], pvt2[:])
                for t in range(NT):
                    tb = slice(t * 128, (t + 1) * 128)
                    atp = pst.tile([128, 97], F32, tag="tp", name="atp_t")
                    nc.tensor.transpose(atp[:], pvs2[:, tb], idf32[0:97, 0:97])
                    for h, srow in ((hA, 0), (hD, 64)):
                        rc = wrk.tile([128, 1], F32, tag="rc", name="rc_t", bufs=4)
                        nc.vector.reciprocal(rc[:], atp[:, srow + 32 : srow + 33])
                        ob = outsb[t][:, h * HD : (h + 1) * HD]
                        av = atp[:, srow : srow + 32]
                        if fin_rr % 16 == 0:
                            nc.scalar.activation(ob, av, AF.Relu, scale=rc[:])
                        elif fin_rr % 2 == 1:
                            nc.gpsimd.tensor_scalar(ob, av, rc[:], 0.0, AL.mult, AL.max)
                        else:
                            nc.vector.tensor_scalar(ob, av, rc[:], 0.0, AL.mult, AL.max)
                        fin_rr += 1

            for t in range(NT):
                nc.sync.dma_start(out_d[t * 128 : (t + 1) * 128, :], outsb[t][:])

    nc.compile()
    return nc


def get_nc():
    if "nc" not in _CACHE:
        _CACHE["nc"] = _build_bass()
    return _CACHE["nc"]


def host_prep(node_features, adjacency, W, a, bias):
    node_features = np.asarray(node_features, dtype=np.float32)
    adjacency = np.asarray(adjacency)
    W = np.asarray(W, dtype=np.float32)
    a = np.asarray(a, dtype=np.float32)
    bias = np.asarray(bias, dtype=np.float32)

    wcat = np.zeros((D + 1, WC), np.float32)
    for h in range(H):
        wcat[:D, h * 33 : h * 33 + HD] = W[h]
        wcat[D, h * 33 : h * 33 + HD] = bias[h * HD : (h + 1) * HD]
        wcat[D, h * 33 + HD] = 1.0  # ones column via x-aug row
        wcat[:D, H * (HD + 1) + h] = W[h] @ a[h, :HD]  # e1 (query) term
        wcat[:D, H * (HD + 1) + H + h] = W[h] @ a[h, HD:]  # e2 (source) term
    wcat16 = wcat.astype(np.float16)

    in_maps = []
    for b in range(B):
        xT = np.ones((D + 1, N), np.float16)
        xT[:D] = node_features[b].T
        adjT = np.ascontiguousarray(adjacency[b].T).astype(np.float16)
        nadj = ((adjT - 1.0) * (-MASKV)).astype(np.float16)
        in_maps.append({"xT": xT, "adjT": adjT, "nadj": nadj, "wc": wcat16})
    return in_maps


def kernel(node_features, adjacency, W, a, bias):
    nc = get_nc()
    in_maps = host_prep(node_features, adjacency, W, a, bias)
    res = run_bass_kernel_spmd(nc, in_maps, core_ids=list(range(B)))
    return np.stack([res.results[b]["out"] for b in range(B)], axis=0)


# revision 9
# speedup vs baseline: 1.1400x; 1.1400x over previous
"""Trainium2 Bass kernel for the CustomGATLayer problem.

Strategy: data-parallel over batch B=8 across the 8 NeuronCores (one batch
element per core).  Per core the masked GAT attention

    P[j,i] = adj_T[j,i] * exp(LeakyReLU(e1[i] + e2[j], 0.2))

is computed in a transposed layout (source node j on partitions, query node
i on the free axis) via two complementary routes, chosen per (head, j-tile)
to balance engine load:

* Route A (PE+ACT):  the PE builds  S = 1 (x) e1  +  I @ (-60*(1-adj_T))
  directly in PSUM (rank-1 broadcast matmul plus an identity-stationary
  "mask add" matmul), so the adjacency mask is folded additively into the
  scores at zero vector-engine cost.  The ACT engine then applies
  Prelu(S + e2[j], alpha=.2) and Exp.  Masked entries come out as
  exp(.2*(s-60)) ~ 3e-5, which is negligible against typical row masses.

* Route D (DVE/Pool):  exp factorisation  P = adj_T * max(exp(e1)exp(e2),
  exp(.2 e1)exp(.2 e2)) using exp'd rank-1 factors: a DMA-broadcast row
  tensor V1b/V2b (from a DRAM round-trip) times per-partition scalars, a
  tensor_tensor max (mostly on GPSIMD/Pool) and the adjacency mask multiply
  (DVE).

The ones-column and the output bias are folded into the Wh matmul by
augmenting x with a ones row and W with a bias row, so the softmax
denominator rides along as matmul column 32 and the epilogue is a single
fused (x*recip) relu-max per (head, tile).  Two heads (one per route) are
processed as a pair sharing a [66, N] PSUM accumulator; the epilogue
transposes both heads' results in one PE transpose.
"""

import numpy as np

import concourse.bacc as bacc
import concourse.bass as bass
import concourse.mybir as mybir
import concourse.tile as tile
from concourse.bass_utils import run_bass_kernel_spmd
from concourse.masks import make_identity

B, N, D, H, HD = 8, 1024, 256, 8, 32
NT = N // 128  # node tiles per core
WC = H * (HD + 1) + 2 * H  # 280 = 264 wh(+ones) cols + 8 e1 cols + 8 e2 cols
F32 = mybir.dt.float32
F16 = mybir.dt.float16
AL = mybir.AluOpType
AF = mybir.ActivationFunctionType

# (A-route head, D-route head, #A-tiles of the A head) per processing pair.
PAIRS = [(5, 0, 7), (6, 1, 7), (7, 2, 6), (4, 3, 6)]

MASKV = -60.0  # additive mask: exp(0.2*(s-60)) ~ 0 at f16

_CACHE: dict = {}


def _build_bass():
    nc = bacc.Bacc("TRN2", target_bir_lowering=False, debug=False, num_devices=B)

    xT_d = nc.dram_tensor("xT", [D + 1, N], F16, kind="ExternalInput")
    wc_d = nc.dram_tensor("wc", [D + 1, WC], F16, kind="ExternalInput")
    adjT_d = nc.dram_tensor("adjT", [N, N], F16, kind="ExternalInput")
    nadj_d = nc.dram_tensor("nadj", [N, N], F16, kind="ExternalInput")
    out_d = nc.dram_tensor("out", [N, H * HD], F32, kind="ExternalOutput")
    vsc_d = nc.dram_tensor("vsc", [2 * H, N], F16, kind="Internal")

    with tile.TileContext(nc) as tc:
        with (
            tc.tile_pool(name="cst", bufs=1) as cst,
            tc.tile_pool(name="wrk", bufs=3) as wrk,
            tc.tile_pool(name="pp", bufs=8) as pp,
            tc.tile_pool(name="pst", bufs=2, space=bass.MemorySpace.PSUM) as pst,
            tc.tile_pool(name="psS", bufs=2, space=bass.MemorySpace.PSUM) as psS,
            tc.tile_pool(name="psV", bufs=1, space=bass.MemorySpace.PSUM) as psV,
        ):
            xt = [cst.tile([128, N], F16, tag=f"xt{k}", name=f"xt{k}") for k in range(2)]
            xta = cst.tile([1, N], F16, tag="xta", name="xta")
            wc = [cst.tile([128, WC], F16, tag=f"wc{k}", name=f"wc{k}") for k in range(2)]
            wca = cst.tile([1, WC], F16, tag="wca", name="wca")
            adjT = [cst.tile([128, N], F16, tag=f"adj{j}", name=f"adj{j}") for j in range(NT)]
            nadj = [cst.tile([128, N], F16, tag=f"nad{j}", name=f"nad{j}") for j in range(NT)]
            whb = [cst.tile([128, H * (HD + 1)], F16, tag=f"whb{j}", name=f"whb{j}") for j in range(NT)]
            e12a = cst.tile([128, 16 * NT], F32, tag="e12a", name="e12a")
            e2a = cst.tile([128, H * NT], F32, tag="e2a", name="e2a")
            ue1 = cst.tile([128, H * NT], F32, tag="ue1", name="ue1")
            ue2 = cst.tile([128, H * NT], F32, tag="ue2", name="ue2")
            eiT16 = cst.tile([H, N], F16, tag="eiT16", name="eiT16")
            e1row = [cst.tile([1, N], F16, tag=f"e1r{h}", name=f"e1r{h}") for h in range(H)]
            v1T = cst.tile([H, N], F16, tag="v1T", name="v1T")
            v2T = cst.tile([H, N], F16, tag="v2T", name="v2T")
            V1b = [cst.tile([128, N], F16, tag=f"V1b{h}", name=f"V1b{h}") for h in range(H)]
            V2b = [cst.tile([128, N], F16, tag=f"V2b{h}", name=f"V2b{h}") for h in range(H)]
            idf16 = cst.tile([128, 128], F16, tag="idf16", name="idf16")
            idf32 = cst.tile([128, 128], F32, tag="idf32", name="idf32")
            ones_r = cst.tile([1, 128], F16, tag="ones_r", name="ones_r")
            outsb = [cst.tile([128, H * HD], F32, tag=f"o{t}", name=f"o{t}") for t in range(NT)]

            # ---- input loads.  nadj on the Pool queue (fast issue; needed
            # from the first A-tile), the rest on SP.
            for k in range(2):
                nc.sync.dma_start(xt[k][:], xT_d[k * 128 : (k + 1) * 128, :])
                nc.sync.dma_start(wc[k][:], wc_d[k * 128 : (k + 1) * 128, :])
            nc.sync.dma_start(xta[:], xT_d[D : D + 1, :])
            nc.sync.dma_start(wca[:], wc_d[D : D + 1, :])
            for j in range(NT):
                nc.sync.dma_start(nadj[j][:], nadj_d[j * 128 : (j + 1) * 128, :])
            make_identity(nc, idf16[:])
            make_identity(nc, idf32[:])
            nc.gpsimd.memset(ones_r[:], 1.0)

            # ---- per-head linear transform Wh (+ones col, +bias via x-aug)
            # and attention logit columns e1/e2.
            for t in range(NT):
                tb = slice(t * 128, (t + 1) * 128)
                whp = pst.tile([128, WC], F32, tag="tp", name="whp_t")
                nc.tensor.matmul(whp[:], xt[0][:, tb], wc[0][:], start=True, stop=False)
                nc.tensor.matmul(whp[:], xt[1][:, tb], wc[1][:], start=False, stop=False)
                nc.tensor.matmul(whp[:], xta[0:1, tb], wca[:], start=False, stop=True)
                nc.vector.tensor_copy(whb[t][:], whp[:, 0 : H * (HD + 1)])
                nc.vector.tensor_copy(e12a[:, t * 16 : (t + 1) * 16], whp[:, H * (HD + 1) : WC])
                nc.vector.tensor_copy(e2a[:, t * 8 : (t + 1) * 8], whp[:, H * (HD + 1) + H : WC])

            # exp'd e2 factors (per-partition scalars for route D)
            nc.scalar.activation(ue1[:], e2a[:], AF.Exp)
            nc.scalar.activation(ue2[:], e2a[:], AF.Exp, scale=0.2)

            # e1 rows: transpose per tile, keep f16 row form
            for t in range(NT):
                etp = pst.tile([16, 128], F32, tag="tp", name="etp_t")
                nc.tensor.transpose(etp[:], e12a[:, t * 16 : (t + 1) * 16], idf32[:])
                nc.gpsimd.tensor_copy(eiT16[:, t * 128 : (t + 1) * 128], etp[0:H, :])

            # stage e1 rows at base partition 0 (PE moving operand requirement)
            for h in range(H):
                nc.scalar.dma_start(e1row[h][:], eiT16[h : h + 1, :])

            # exp'd e1 rows -> DRAM -> partition-broadcast tiles (route D)
            nc.scalar.activation(v1T[:], eiT16[:], AF.Exp)
            nc.scalar.activation(v2T[:], eiT16[:], AF.Exp, scale=0.2)
            nc.sync.dma_start(vsc_d[0:H, :], v1T[:])
            nc.sync.dma_start(vsc_d[H : 2 * H, :], v2T[:])
            for pi, (hA, hD, _) in enumerate(PAIRS):
                for h in (hD, hA):
                    nc.sync.dma_start(V1b[h][:], vsc_d[h : h + 1, :].partition_broadcast(128))
                    nc.sync.dma_start(V2b[h][:], vsc_d[H + h : H + h + 1, :].partition_broadcast(128))
                for j in (2 * pi, 2 * pi + 1):
                    nc.sync.dma_start(adjT[j][:], adjT_d[j * 128 : (j + 1) * 128, :])

            # ---- main attention loop: pairs of heads, tile-interleaved ----
            fin_rr = 0  # round-robin for the epilogue engine

            for pi, (hA, hD, nA) in enumerate(PAIRS):
                pvt2 = psV.tile([97, N], F32, tag="pv", name="pvt2_t")
                pend = []  # PV matmuls deferred one j-round for PE flow

                def flush_pv(pend=pend):
                    for h, srow, j, P in pend:
                        for half in range(2):
                            sl = slice(half * 512, (half + 1) * 512)
                            nc.tensor.matmul(
                                pvt2[srow : srow + 33, sl],
                                whb[j][:, h * 33 : (h + 1) * 33],
                                P[:, sl],
                                start=(j == 0),
                                stop=(j == NT - 1),
                            )
                    pend.clear()

                for j in range(NT):
                    prev = list(pend)
                    pend.clear()
                    for h, srow in ((hA, 0), (hD, 64)):
                        bias_ap = e12a[:, j * 16 + 8 + h : j * 16 + 9 + h]
                        if srow == 0 and j < nA:
                            # Route A: PE score build + ACT prelu/exp
                            S = psS.tile([128, N], F32, tag="S", name="S_t")
                            for half in range(2):
                                sl = slice(half * 512, (half + 1) * 512)
                                nc.tensor.matmul(
                                    S[:, sl], ones_r[:], e1row[h][0:1, sl],
                                    start=True, stop=False,
                                )
                                nc.tensor.matmul(
                                    S[:, sl], idf16[:], nadj[j][:, sl],
                                    start=False, stop=True,
                                )
                            lr = wrk.tile([128, N], F16, tag="lr", name="lr_t", bufs=3)
                            nc.scalar.activation(lr[:], S[:], AF.Prelu, bias=bias_ap, alpha=0.2)
                            P = pp.tile([128, N], F16, tag="P", name="P_t")
                            nc.scalar.activation(P[:], lr[:], AF.Exp)
                        else:
                            # Route D: exp'd rank-1 factors + max + mask
                            E1 = wrk.tile([128, N], F16, tag="E1", name="E1_t", bufs=3)
                            E2 = wrk.tile([128, N], F16, tag="E2", name="E2_t", bufs=3)
                            Pp = wrk.tile([128, N], F16, tag="Pp", name="Pp_t", bufs=3)
                            s1 = ue1[:, j * 8 + h : j * 8 + h + 1]
                            s2 = ue2[:, j * 8 + h : j * 8 + h + 1]
                            nc.vector.tensor_scalar(E1[:], V1b[h][:], s1, None, AL.mult)
                            nc.vector.tensor_scalar(E2[:], V2b[h][:], s2, None, AL.mult)
                            eng = nc.gpsimd if srow == 64 else nc.vector
                            eng.tensor_tensor(Pp[:], E1[:], E2[:], AL.max)
                            P = pp.tile([128, N], F16, tag="P", name="P_t")
                            nc.vector.tensor_tensor(P[:], Pp[:], adjT[j][:], AL.mult)
                        pend.append((h, srow, j, P))
                    # emit previous round's PV matmuls (keeps PE fed without
                    # head-of-line blocking on this round's P tiles)
                    for h, srow, jj, P in prev:
                        for half in range(2):
                            sl = slice(half * 512, (half + 1) * 512)
                            nc.tensor.matmul(
                                pvt2[srow : srow + 33, sl],
                                whb[jj][:, h * 33 : (h + 1) * 33],
                                P[:, sl],
                                start=(jj == 0),
                                stop=(jj == NT - 1),
                            )
                flush_pv()

                # ---- pair epilogue: drain accumulator, transpose, normalize
                pvs2 = wrk.tile([97, N], F32, tag="pvs", name="pvs_t", bufs=2)
                if pi == 0:
                    nc.gpsimd.tensor_copy(pvs2[:], pvt2[:])
                elif pi == 2:
                    nc.scalar.copy(pvs2[:], pvt2[:])
                else:
                    nc.vector.tensor_copy(pvs2[:], pvt2[:])
                for t in range(NT):
                    tb = slice(t * 128, (t + 1) * 128)
                    atp = pst.tile([128, 97], F32, tag="tp", name="atp_t")
                    nc.tensor.transpose(atp[:], pvs2[:, tb], idf32[0:97, 0:97])
                    for h, srow in ((hA, 0), (hD, 64)):
                        rc = wrk.tile([128, 1], F32, tag="rc", name="rc_t", bufs=4)
                        nc.vector.reciprocal(rc[:], atp[:, srow + 32 : srow + 33])
                        ob = outsb[t][:, h * HD : (h + 1) * HD]
                        av = atp[:, srow : srow + 32]
                        if fin_rr % 16 == 0:
                            nc.scalar.activation(ob, av, AF.Relu, scale=rc[:])
                        elif fin_rr % 2 == 1:
                            nc.gpsimd.tensor_scalar(ob, av, rc[:], 0.0, AL.mult, AL.max)
                        else:
                            nc.vector.tensor_scalar(ob, av, rc[:], 0.0, AL.mult, AL.max)
                        fin_rr += 1

            for t in range(NT):
                nc.sync.dma_start(out_d[t * 128 : (t + 1) * 128, :], outsb[t][:])

    nc.compile()
    return nc


def get_nc():
    if "nc" not in _CACHE:
        _CACHE["nc"] = _build_bass()
    return _CACHE["nc"]


def host_prep(node_features, adjacency, W, a, bias):
    node_features = np.asarray(node_features, dtype=np.float32)
    adjacency = np.asarray(adjacency)
    W = np.asarray(W, dtype=np.float32)
    a = np.asarray(a, dtype=np.float32)
    bias = np.asarray(bias, dtype=np.float32)

    wcat = np.zeros((D + 1, WC), np.float32)
    for h in range(H):
        wcat[:D, h * 33 : h * 33 + HD] = W[h]
        wcat[D, h * 33 : h * 33 + HD] = bias[h * HD : (h + 1) * HD]
        wcat[D, h * 33 + HD] = 1.0  # ones column via x-aug row
        wcat[:D, H * (HD + 1) + h] = W[h] @ a[h, :HD]  # e1 (query) term
        wcat[:D, H * (HD + 1) + H + h] = W[h] @ a[h, HD:]  # e2 (source) term
    wcat16 = wcat.astype(np.float16)

    in_maps = []
    for b in range(B):
        xT = np.ones((D + 1, N), np.float16)
        xT[:D] = node_features[b].T
        adjT = np.ascontiguousarray(adjacency[b].T).astype(np.float16)
        nadj = ((adjT - 1.0) * (-MASKV)).astype(np.float16)
        in_maps.append({"xT": xT, "adjT": adjT, "nadj": nadj, "wc": wcat16})
    return in_maps


def kernel(node_features, adjacency, W, a, bias):
    nc = get_nc()
    in_maps = host_prep(node_features, adjacency, W, a, bias)
    res = run_bass_kernel_spmd(nc, in_maps, core_ids=list(range(B)))
    return np.stack([res.results[b]["out"] for b in range(B)], axis=0)
